# revision 29
# baseline (speedup 1.0000x reference)
"""Trainium2 Bass kernel for nn_CRF mean-field iteration (dense CRF, 5 iters).

Problem (hardcoded shapes): log_unary [1,4,32,16,16], features_pairwise
[1,2,32,16,16], compatibility = Potts (ones - eye).  N = 8192 voxels, C = 4.

Strategy
--------
Per reference, each iteration applies two dense [N,N] Gaussian kernels:
  K1 (bilateral, 5-D features) and K2 (spatial, 3-D features), both with
  rsqrt(rowsum) symmetric normalization, then a Potts compatibility
  transform and a softmax.

Key facts exploited:
  * Potts update: softmax over c is invariant to the per-voxel colsum term,
    so the compatibility transform reduces to a sign flip + unary add.
  * K2 is a Kronecker product of 1-D Gaussians; applied fully on-chip as
    Kronecker-factor matmuls plus two PE transposes.
  * K1 = exp(f.f' -.5|f|^2 -.5|f'|^2): BOTH quadratic terms ride as
    hi/lo-split bf16 rows of the d^2 matmul (K=19), so the exp ACT needs
    no bias and runs 2 tiles per instruction ((N+352)/1.2 ns amortized) -
    the ACT engine streams the whole materialization back-to-back.
  * K1 block rowsum partials via per-2-tile DVE free-dim reduces on the
    fp8 A tiles (DVE is idle during materialization), one 32KB AllGather
    of the per-core partial grids, then a 7-add tree.  s1n (own-block
    columns) is extracted with a per-core one-hot selector matmul.
  * Per-iteration K1 matvec streams A as the fp8 MOVING operand with
    perf_mode=DoubleRow (2 fp8 rows/lane/cycle, measured 216ns per
    [128,2,512] matmul) against a tiny stationary q [128,2,4].
  * ONE 8KB AllGather per iteration in partition-major block layout: the
    gathered buffer lands in exactly the SBUF layout the matvec and the
    separable pipeline need - no transposes on the consumer side.  A 16x
    scale folded into s1m keeps the fp8 q operand in normal range.

Sharding: voxel dim N row-blocked over 8 cores.  Each core materializes and
keeps its [8192 x 1024] column-block of K1 (fp8, 8 MB) in SBUF.
"""

import numpy as np
import ml_dtypes

BF16 = ml_dtypes.bfloat16

B, C, X, Y, Z = 1, 4, 32, 16, 16
N = X * Y * Z            # 8192
P = 128                  # SBUF partitions
NCORES = 8
NB = N // NCORES         # 1024 rows per core
TM = N // P              # 64 m-tiles
TB = NB // P             # 8 block tiles
QS = 16                  # padded per-tile stride of the fp8 q operand
ALPHA = 5.0              # = BETA = GAMMA in this problem
NUM_ITER = 5
W_1 = 1.0
W_2 = 1.0
LN16 = float(np.log(16.0))

_CACHE = {}
DUMMY_AG = True


def _split_hi_lo(v):
    hi = v.astype(BF16).astype(np.float32)
    lo = (v - hi).astype(BF16).astype(np.float32)
    return hi, lo


def _to_block_layout(v_nc):
    """[N, C] -> [NCORES, 128, TB*C] block-p-major device layout."""
    # n = k*NB + tt*128 + p
    return (
        v_nc.reshape(NCORES, TB, P, C).transpose(0, 2, 1, 3).reshape(NCORES, P, TB * C)
    )


def _host_constants(log_unary, features_pairwise):
    """All host-side numpy prep: layouts, constants, initial softmax."""
    lu = np.asarray(log_unary, np.float32).reshape(C, N)
    img = np.asarray(features_pairwise, np.float32).reshape(2, N)

    gx, gy, gz = np.meshgrid(
        np.arange(X), np.arange(Y), np.arange(Z), indexing="ij"
    )
    spatial = np.stack([gx, gy, gz], 0).astype(np.float32).reshape(3, N)

    f1 = np.concatenate([spatial, img], 0) / ALPHA      # [5, N]
    sq1 = (f1 * f1).sum(0)                              # [N]
    bcol = -0.5 * sq1

    f_hi, f_lo = _split_hi_lo(f1)
    b_hi, b_lo = _split_hi_lo(bcol)
    ones = np.ones((1, N), np.float32)
    # row r of lhs multiplies row r of rhs; sum over rows gives the full
    # exponent f_m.f_n - .5|f_n|^2 - .5|f_m|^2 (no ACT bias needed)
    lhs_rows = np.concatenate(
        [f_hi, f_lo, f_hi, ones, ones, b_hi[None], b_lo[None]], 0
    ).astype(BF16)                                      # [19, N]
    rhs_rows = np.concatenate(
        [f_hi, f_hi, f_lo, b_hi[None], b_lo[None], ones, ones], 0
    ).astype(BF16)                                      # [19, NB]

    # initial q0 = softmax(lu), shipped in the AllGather block layout
    # (partition-major: flat index = k*P*TB*C + p*TB*C + tt*C + c)
    e = np.exp(lu - lu.max(0, keepdims=True))
    q0 = (e / e.sum(0, keepdims=True)).T                # [N, 4]
    q0_blk = _to_block_layout(q0).reshape(-1).astype(BF16)

    # separable spatial kernel, normalization + W_2 folded into factors
    def g1d(n):
        a = np.arange(n, dtype=np.float32) / ALPHA
        return np.exp(-0.5 * (a[:, None] - a[None, :]) ** 2)

    Gx, Gy, Gz = g1d(X), g1d(Y), g1d(Z)
    gxp = Gx * (Gx.sum(1) ** -0.5)[:, None] * (Gx.sum(1) ** -0.5)[None, :]
    gyp = Gy * (Gy.sum(1) ** -0.5)[:, None] * (Gy.sum(1) ** -0.5)[None, :]
    gzp = Gz * (Gz.sum(1) ** -0.5)[:, None] * (Gz.sum(1) ** -0.5)[None, :]
    gxp *= W_2

    # Kronecker-factor constants for the on-chip pipeline
    kz = np.kron(np.eye(8, dtype=np.float32), gzp)             # [128, 128]
    ky = np.zeros((P, 4 * P), np.float32)                      # [(h*2+h')*128]
    for h in range(2):
        for hp in range(2):
            blk = np.kron(gyp[h * 8 : (h + 1) * 8, hp * 8 : (hp + 1) * 8],
                          np.eye(16, dtype=np.float32))
            ky[:, (h * 2 + hp) * P : (h * 2 + hp + 1) * P] = blk
    identity = np.eye(P, dtype=np.float32)

    lut_all = _to_block_layout(lu.T)                           # [8, 128, 32]

    in_maps = []
    for k in range(NCORES):
        blk = slice(k * NB, (k + 1) * NB)
        kx = np.kron(gxp[:, 4 * k : 4 * k + 4], np.eye(C, dtype=np.float32))
        ek = np.zeros((TM, TB), np.float32)
        for tt in range(TB):
            ek[k * TB + tt, tt] = 1.0
        in_maps.append(
            {
                "lhs_rows": np.ascontiguousarray(lhs_rows),
                "rhs_rows": np.ascontiguousarray(rhs_rows[:, blk]),
                "lut": np.ascontiguousarray(lut_all[k]),
                "q0": q0_blk,
                "kz": kz.astype(BF16),
                "ky": ky.astype(BF16),
                "kx": kx.astype(BF16),                         # [128, 16]
                "idb": identity.astype(BF16),
                "idf": identity,
                "ek": ek,
                "lncst": np.tile(
                    np.array([[LN16, -LN16]], np.float32), (P, 1)
                ),
            }
        )
    return in_maps


def _build_program():
    """Build the SPMD Bass/Tile program (same NEFF on all 8 cores)."""
    import concourse.bacc as bacc
    import concourse.mybir as mybir
    import concourse.tile as tile

    f32 = mybir.dt.float32
    bf16 = mybir.dt.bfloat16
    fp8 = mybir.dt.float8e4
    AF = mybir.ActivationFunctionType
    ln_func = getattr(AF, "Ln", None) or getattr(AF, "Log")
    DR = mybir.MatmulPerfMode.DoubleRow
    RG = [list(range(NCORES))]

    nc = bacc.Bacc(
        "TRN2", target_bir_lowering=False, debug=False, num_devices=NCORES
    )

    # I/O
    lhs_rows = nc.dram_tensor("lhs_rows", [19, N], bf16, kind="ExternalInput")
    rhs_rows = nc.dram_tensor("rhs_rows", [19, NB], bf16, kind="ExternalInput")
    lut_in = nc.dram_tensor("lut", [P, TB * C], f32, kind="ExternalInput")
    q0_in = nc.dram_tensor("q0", [NCORES * P * TB * C], bf16, kind="ExternalInput")
    kz_in = nc.dram_tensor("kz", [P, P], bf16, kind="ExternalInput")
    ky_in = nc.dram_tensor("ky", [P, 4 * P], bf16, kind="ExternalInput")
    kx_in = nc.dram_tensor("kx", [P, 4 * C], bf16, kind="ExternalInput")
    idb_in = nc.dram_tensor("idb", [P, P], bf16, kind="ExternalInput")
    idf_in = nc.dram_tensor("idf", [P, P], f32, kind="ExternalInput")
    ek_in = nc.dram_tensor("ek", [TM, TB], f32, kind="ExternalInput")
    lncst_in = nc.dram_tensor("lncst", [P, 2], f32, kind="ExternalInput")
    qout = nc.dram_tensor("qout", [P, TB * C], f32, kind="ExternalOutput")

    with tile.TileContext(nc) as tc:
        with (
            tc.tile_pool(name="const", bufs=1) as cp,
            tc.tile_pool(name="dram", bufs=1, space="DRAM") as dp,
        ):
            # ---- persistent SBUF tensors ----
            A_sb = cp.tile([P, TM * NB], fp8, name="A_sb")        # 64 KB/part
            lhsr_sb = cp.tile([19, N], bf16, name="lhsr_sb")
            rhsr_sb = cp.tile([19, NB], bf16, name="rhsr_sb")
            lut_sb = cp.tile([P, TB * C], f32, name="lut_sb")
            prs = cp.tile([P, TM], f32, name="prs")
            rs8_sb = cp.tile([P, NCORES * TM], f32, name="rs8_sb")
            rs_sb = cp.tile([P, TM], f32, name="rs_sb")
            kz_sb = cp.tile([P, P], bf16, name="kz_sb")
            ky_sb = cp.tile([P, 4 * P], bf16, name="ky_sb")
            kx_sb = cp.tile([P, 4 * C], bf16, name="kx_sb")
            idb_sb = cp.tile([P, P], bf16, name="idb_sb")
            idf_sb = cp.tile([P, P], f32, name="idf_sb")
            ek_sb = cp.tile([TM, TB], f32, name="ek_sb")
            lncst_sb = cp.tile([P, 2], f32, name="lncst_sb")

            A3 = A_sb[:].rearrange("p (t n) -> p t n", n=NB)

            # ---- DRAM scratch ----
            dum_in = dp.tile([512], f32, name="dum_in")
            dum_out = dp.tile([4096], f32, name="dum_out", addr_space="Shared")
            rs_loc = dp.tile([P * TM], f32, name="rs_loc")
            rs_all = dp.tile(
                [NCORES * P * TM], f32, name="rs_all", addr_space="Shared"
            )
            qag_in = [
                dp.tile([P * TB * C], bf16, name=f"qag_in{i}") for i in range(4)
            ]
            qag_out = [
                dp.tile(
                    [NCORES * P * TB * C], bf16, name=f"qag_out{i}",
                    addr_space="Shared",
                )
                for i in range(4)
            ]

            # ---- dummy collective first: hides the one-time global
            #      collective-entry barrier under materialization ----
            if DUMMY_AG:
                nc.sync.dma_start(
                    out=dum_in[:], in_=idf_in.ap().rearrange("p q -> (p q)")[0:512]
                )
                nc.gpsimd.collective_compute(
                    "AllGather",
                    mybir.AluOpType.bypass,
                    replica_groups=RG,
                    ins=[dum_in[:]],
                    outs=[dum_out[:]],
                )

            # ---- load constants; d^2 inputs first, with a small head
            #      chunk of lhs so the first matmuls start early ----
            nc.sync.dma_start(
                out=rhsr_sb[:, 0:512], in_=rhs_rows.ap()[:, 0:512]
            )
            nc.sync.dma_start(
                out=lhsr_sb[:, 0:256], in_=lhs_rows.ap()[:, 0:256]
            )
            nc.sync.dma_start(
                out=rhsr_sb[:, 512:1024], in_=rhs_rows.ap()[:, 512:1024]
            )
            nc.sync.dma_start(
                out=lhsr_sb[:, 256:2048], in_=lhs_rows.ap()[:, 256:2048]
            )
            for ch in range(1, 4):
                nc.sync.dma_start(
                    out=lhsr_sb[:, ch * 2048 : (ch + 1) * 2048],
                    in_=lhs_rows.ap()[:, ch * 2048 : (ch + 1) * 2048],
                )
            nc.scalar.dma_start(out=lut_sb[:], in_=lut_in.ap())
            nc.scalar.dma_start(out=kz_sb[:], in_=kz_in.ap())
            nc.scalar.dma_start(out=ky_sb[:], in_=ky_in.ap())
            nc.scalar.dma_start(out=kx_sb[:], in_=kx_in.ap())
            nc.scalar.dma_start(out=idb_sb[:], in_=idb_in.ap())
            nc.scalar.dma_start(out=idf_sb[:], in_=idf_in.ap())
            nc.scalar.dma_start(out=ek_sb[:], in_=ek_in.ap())
            nc.scalar.dma_start(out=lncst_sb[:], in_=lncst_in.ap())
            warm_exp = cp.tile([P, 1], f32, name="warm_exp")
            nc.scalar.activation(warm_exp[:], lncst_sb[:, 0:1], AF.Exp)

            # ================= materialization of A = K1 block =============
            # d^2 matmul (full exponent incl. both quadratic terms) -> exp on
            # ACT, two tiles per instruction; block rowsum partials via DVE
            # free-dim reduces on the fp8 A tiles (DVE is otherwise idle).
            with tc.tile_pool(name="matps", bufs=2, space="PSUM") as matps:
                for tq in range(TM // 2):
                    ps = matps.tile([P, 2 * NB], f32, name="mat_ps", tag="mat")
                    for t2 in range(2):
                        t = 2 * tq + t2
                        for h in range(2):
                            nc.tensor.matmul(
                                ps[:, t2 * NB + h * 512 : t2 * NB + (h + 1) * 512],
                                lhsr_sb[:, t * P : (t + 1) * P],
                                rhsr_sb[:, h * 512 : (h + 1) * 512],
                                start=True,
                                stop=True,
                                skip_group_check=True,
                            )
                    nc.scalar.activation(
                        A_sb[:, 2 * tq * NB : (2 * tq + 2) * NB], ps[:], AF.Exp
                    )
                    nc.vector.reduce_sum(
                        prs[:, 2 * tq : 2 * tq + 2],
                        A_sb[
                            :, 2 * tq * NB : (2 * tq + 2) * NB
                        ].rearrange("p (t n) -> p t n", n=NB),
                        axis=mybir.AxisListType.X,
                    )

            # ---- iteration pools open early: iteration 0's q-load and
            #      separable pipeline are emitted BEFORE the rowsum
            #      distribution, so every engine FIFO has runnable iter-0
            #      work queued ahead of the instructions that block on the
            #      rowsum AllGather.
            with (
                tc.tile_pool(name="itp", bufs=2) as itp,
                tc.tile_pool(name="sep", bufs=1) as sepp,
                tc.tile_pool(name="qps", bufs=2, space="PSUM") as qpsp,
                tc.tile_pool(name="sps", bufs=3, space="PSUM") as spsp,
            ):
                def emit_qload(it, q_l):
                    qsrc = q0_in.ap() if it == 0 else qag_out[it - 1][:]
                    ql3 = q_l[:].rearrange("p (k tc) -> p k tc", k=NCORES)
                    src3 = qsrc.rearrange("(k p tc) -> p k tc", k=NCORES, p=P)
                    nc.sync.dma_start(out=ql3[:, 0:4, :], in_=src3[:, 0:4, :])
                    nc.scalar.dma_start(out=ql3[:, 4:8, :], in_=src3[:, 4:8, :])

                def emit_sep(q_l, inter):
                    # separable spatial kernel (K2), all on-chip
                    zp = spsp.tile([P, TM * C], f32, name="zp", tag="sep")
                    nc.tensor.matmul(
                        zp[:], kz_sb[:], q_l[:], start=True, stop=True
                    )
                    w1 = sepp.tile([P, TM * C], bf16, name="w1")
                    nc.vector.tensor_copy(w1[:], zp[:])
                    inter(0)

                    yp = spsp.tile([P, 2 * X * C], f32, name="yp", tag="sep")
                    w1r = w1[:].rearrange("p (x h c) -> p x h c", h=2, c=C)
                    for hp in range(2):
                        for h in range(2):
                            nc.tensor.matmul(
                                yp[:, hp * P : (hp + 1) * P],
                                ky_sb[:, (h * 2 + hp) * P : (h * 2 + hp + 1) * P],
                                w1r[:, :, h, :],
                                start=(h == 0),
                                stop=(h == 1),
                            )
                    w2 = sepp.tile([P, 2 * X * C], bf16, name="w2")
                    nc.vector.tensor_copy(w2[:], yp[:])
                    inter(1)

                    q2sb = sepp.tile([P, TB * C], f32, name="q2sb")
                    q2r = q2sb[:].rearrange("p (x h c) -> p x h c", h=2, c=C)
                    for hp in range(2):
                        tp1 = spsp.tile([P, P], bf16, name="tp1", tag="sep")
                        nc.tensor.transpose(
                            tp1[:], w2[:, hp * P : (hp + 1) * P], idb_sb[:]
                        )
                        tx = sepp.tile([P, P], bf16, name="tx", tag="tx")
                        nc.vector.tensor_copy(tx[:], tp1[:])
                        xp = spsp.tile([4 * C, P], f32, name="xp", tag="sep")
                        nc.tensor.matmul(
                            xp[:], kx_sb[:], tx[:], start=True, stop=True
                        )
                        sx = sepp.tile([4 * C, P], bf16, name="sx", tag="sx")
                        nc.vector.tensor_copy(sx[:], xp[:])
                        tp2 = spsp.tile([P, 4 * C], bf16, name="tp2", tag="sep")
                        nc.tensor.transpose(
                            tp2[:], sx[:], idb_sb[:4 * C, :4 * C]
                        )
                        nc.vector.tensor_copy(
                            q2r[:, :, hp, :],
                            tp2[:].rearrange("p (x c) -> p x c", c=C),
                        )
                        inter(2 + hp)
                    # logits base = lu + q2, off the critical path
                    lq2 = sepp.tile([P, TB * C], f32, name="lq2")
                    nc.vector.tensor_add(lq2[:], q2sb[:], lut_sb[:])
                    return lq2

                # -- iteration-0 prefetch (no AllGather dependency) --
                q_l0 = itp.tile([P, TM * C], bf16, name="q_l", tag="q_l")
                emit_qload(0, q_l0)

                # -- rowsum distribution: AllGather partial grids --
                nc.sync.dma_start(
                    out=rs_loc[:].rearrange("(p t) -> p t", t=TM), in_=prs[:]
                )
                nc.gpsimd.collective_compute(
                    "AllGather",
                    mybir.AluOpType.bypass,
                    replica_groups=RG,
                    ins=[rs_loc[:]],
                    outs=[rs_all[:]],
                )
                nc.scalar.dma_start(
                    out=rs8_sb[:].rearrange("p (j t) -> p j t", t=TM),
                    in_=rs_all[:].rearrange("(j p t) -> p j t", p=P, t=TM),
                )

                # iteration-0 separable pipeline fills the AllGather wait
                lq2_0 = emit_sep(q_l0, lambda s: None)

                # -- 7-add tree (emitted after the sep copies so the blocked
                #    adds don't head-of-line block the Vector FIFO) --
                r8 = rs8_sb[:].rearrange("p (j t) -> p j t", t=TM)
                nc.vector.tensor_add(r8[:, 0, :], r8[:, 0, :], r8[:, 1, :])
                nc.vector.tensor_add(r8[:, 2, :], r8[:, 2, :], r8[:, 3, :])
                nc.vector.tensor_add(r8[:, 4, :], r8[:, 4, :], r8[:, 5, :])
                nc.vector.tensor_add(r8[:, 6, :], r8[:, 6, :], r8[:, 7, :])
                nc.vector.tensor_add(r8[:, 0, :], r8[:, 0, :], r8[:, 2, :])
                nc.vector.tensor_add(r8[:, 4, :], r8[:, 4, :], r8[:, 6, :])
                nc.vector.tensor_add(rs_sb[:], r8[:, 0, :], r8[:, 4, :])

                # S1 = rs^-1/2 via exp(-0.5*ln(rs)); 16x folded into s1m,
                # 1/16 into s1n.  Ln/Ln then Exp/Exp: two table switches.
                s1m_raw = cp.tile([P, TM], f32, name="s1m_raw")
                s1m_1 = cp.tile([P, TM], bf16, name="s1m_1")
                nc.scalar.activation(s1m_raw[:], rs_sb[:], ln_func)

                # s1n: own-block columns of rs via the one-hot selector ek
                rtp = spsp.tile([TM, P], f32, name="rtp", tag="sep")
                nc.tensor.transpose(rtp[:], rs_sb[:], idf_sb[:])
                rsT_sb = cp.tile([TM, P], f32, name="rsT_sb")
                nc.vector.tensor_copy(rsT_sb[:], rtp[:])
                snT = spsp.tile([TB, P], f32, name="snT", tag="sep")
                nc.tensor.matmul(
                    snT[:], ek_sb[:], rsT_sb[:], start=True, stop=True
                )
                snT_sb = cp.tile([TB, P], f32, name="snT_sb")
                nc.vector.tensor_copy(snT_sb[:], snT[:])
                snr = spsp.tile([P, TB], f32, name="snr", tag="sep")
                nc.tensor.transpose(snr[:], snT_sb[:], idf_sb[:TB, :TB])
                s1n_raw = cp.tile([P, TB], f32, name="s1n_raw")
                s1n_1 = cp.tile([P, TB], f32, name="s1n_1")
                nc.scalar.activation(s1n_raw[:], snr[:], ln_func)
                nc.scalar.activation(
                    s1m_1[:], s1m_raw[:], AF.Exp, scale=-0.5,
                    bias=lncst_sb[:, 0:1],
                )
                nc.scalar.activation(
                    s1n_1[:], s1n_raw[:], AF.Exp, scale=-0.5,
                    bias=lncst_sb[:, 1:2],
                )
                s1n_rep = cp.tile([P, TB * C], f32, name="s1n_rep")
                s1n_r3 = s1n_rep[:].rearrange("p (t c) -> p t c", c=C)
                for c in range(C):
                    (nc.vector if c < 2 else nc.gpsimd).tensor_copy(
                        s1n_r3[:, :, c], s1n_1[:]
                    )
                s1m_3 = s1m_1[:].rearrange("p (t u) -> p t u", u=1)

                # ======================= iterations =======================
                for it in range(NUM_ITER):
                    last = it == NUM_ITER - 1

                    if it == 0:
                        q_l = q_l0
                    else:
                        q_l = itp.tile([P, TM * C], bf16, name="q_l", tag="q_l")
                        emit_qload(it, q_l)

                    # -- q_s = q_l * (16*s1m)  (fp8, padded stride-16) --
                    q_s = itp.tile([P, TM * QS], fp8, name="q_s", tag="q_s")
                    qs3 = q_s[:].rearrange("p (t c) -> p t c", c=QS)
                    ql4 = q_l[:].rearrange("p (t c) -> p t c", c=C)
                    for c in range(C):
                        eng = nc.vector if c % 2 == 0 else nc.gpsimd
                        eng.tensor_mul(
                            qs3[:, :, c : c + 1], ql4[:, :, c : c + 1], s1m_3
                        )

                    q_ps = [
                        qpsp.tile(
                            [C, 512], f32, name=f"q_ps{h}", tag=f"qps{h}", bufs=1
                        )
                        for h in range(2)
                    ]

                    def matvec(h, tp_range):
                        for tp in tp_range:
                            nc.tensor.matmul(
                                q_ps[h][:],
                                qs3[:, 2 * tp : 2 * tp + 2, 0:C],
                                A3[:, 2 * tp : 2 * tp + 2, h * 512 : (h + 1) * 512],
                                start=(tp == 0),
                                stop=(tp == TM // 2 - 1),
                                perf_mode=DR,
                                skip_group_check=True,
                            )

                    if it == 0:
                        lq2 = lq2_0
                        matvec(0, range(0, 32))
                        matvec(1, range(0, 32))
                    else:
                        matvec(0, range(0, 16))

                        def inter(stage):
                            if stage == 0:
                                matvec(0, range(16, 32))
                            elif stage == 1:
                                matvec(1, range(0, 16))
                            elif stage == 2:
                                matvec(1, range(16, 24))
                            else:
                                matvec(1, range(24, 32))

                        lq2 = emit_sep(q_l, inter)

                    # -- transpose matvec result [4, NB] -> [128, TB*C] --
                    cn_sb = sepp.tile([C, NB], bf16, name="cn_sb")
                    for h in range(2):
                        nc.vector.tensor_copy(
                            cn_sb[:, h * 512 : (h + 1) * 512], q_ps[h][:]
                        )
                    qb_ps = spsp.tile(
                        [P, TB * C], bf16, name="qb_ps", tag="qb", bufs=1
                    )
                    for tt in range(TB):
                        nc.tensor.matmul(
                            qb_ps[:, tt * C : (tt + 1) * C],
                            cn_sb[:, tt * P : (tt + 1) * P],
                            idb_sb[:C, :C],
                            is_transpose=True,
                            start=(tt == 0),
                            stop=(tt == TB - 1),
                            skip_group_check=True,
                        )

                    # -- epilogue: logits = lu + S1n*u1 + q2 ; softmax --
                    u_sb = sepp.tile([P, TB * C], f32, name="u_sb")
                    nc.vector.tensor_mul(u_sb[:], qb_ps[:], s1n_rep[:])
                    nc.vector.tensor_add(u_sb[:], u_sb[:], lq2[:])
                    e_sb = sepp.tile([P, TB * C], f32, name="e_sb")
                    nc.scalar.activation(e_sb[:], u_sb[:], AF.Exp)
                    zs = sepp.tile([P, TB], f32, name="zs")
                    nc.vector.reduce_sum(
                        zs[:],
                        e_sb[:].rearrange("p (t c) -> p t c", c=C),
                        axis=mybir.AxisListType.X,
                    )
                    rz = sepp.tile([P, TB], f32, name="rz")
                    nc.vector.reciprocal(rz[:], zs[:])
                    rz_rep = sepp.tile([P, TB * C], f32, name="rz_rep")
                    rzr3 = rz_rep[:].rearrange("p (t c) -> p t c", c=C)
                    nc.vector.tensor_copy(rzr3[:, :, 0], rz[:])
                    nc.vector.tensor_copy(rzr3[:, :, 1], rz[:])
                    nc.gpsimd.tensor_copy(rzr3[:, :, 2], rz[:])
                    nc.gpsimd.tensor_copy(rzr3[:, :, 3], rz[:])
                    qn = sepp.tile(
                        [P, TB * C], f32 if last else bf16, name="qn",
                        tag="qn_f" if last else "qn_b",
                    )
                    nc.vector.tensor_mul(qn[:], e_sb[:], rz_rep[:])

                    if last:
                        nc.sync.dma_start(out=qout.ap(), in_=qn[:])
                    else:
                        nc.sync.dma_start(
                            out=qag_in[it][:].rearrange("(p tc) -> p tc", p=P),
                            in_=qn[:],
                        )
                        nc.gpsimd.collective_compute(
                            "AllGather",
                            mybir.AluOpType.bypass,
                            replica_groups=RG,
                            ins=[qag_in[it][:]],
                            outs=[qag_out[it][:]],
                        )

    nc.compile()
    return nc


def get_program():
    if "nc" not in _CACHE:
        _CACHE["nc"] = _build_program()
    return _CACHE["nc"]


def kernel(log_unary, features_pairwise, compatibility_weights):
    import concourse.bass_utils as bass_utils

    log_unary = np.asarray(log_unary)
    features_pairwise = np.asarray(features_pairwise)
    compatibility_weights = np.asarray(compatibility_weights)
    assert log_unary.shape == (B, C, X, Y, Z)
    assert features_pairwise.shape == (B, 2, X, Y, Z)
    potts = np.ones((C, C), np.float32) - np.eye(C, dtype=np.float32)
    assert np.abs(compatibility_weights.astype(np.float32) - potts).max() < 1e-5

    in_maps = _host_constants(log_unary, features_pairwise)
    nc = get_program()
    res = bass_utils.run_bass_kernel_spmd(
        nc, in_maps, core_ids=list(range(NCORES))
    )
    # qout[k] is [128, TB*C] block-p-major; invert the layout
    q = np.stack([res.results[k]["qout"] for k in range(NCORES)], 0)
    q = q.reshape(NCORES, P, TB, C).transpose(0, 2, 1, 3).reshape(N, C)
    out = q.T.reshape(B, C, X, Y, Z).astype(np.float32)
    return out


# revision 30
# speedup vs baseline: 1.0024x; 1.0024x over previous
"""Trainium2 Bass kernel for nn_CRF mean-field iteration (dense CRF, 5 iters).

Problem (hardcoded shapes): log_unary [1,4,32,16,16], features_pairwise
[1,2,32,16,16], compatibility = Potts (ones - eye).  N = 8192 voxels, C = 4.

Strategy
--------
Per reference, each iteration applies two dense [N,N] Gaussian kernels:
  K1 (bilateral, 5-D features) and K2 (spatial, 3-D features), both with
  rsqrt(rowsum) symmetric normalization, then a Potts compatibility
  transform and a softmax.

Key facts exploited:
  * Potts update: softmax over c is invariant to the per-voxel colsum term,
    so the compatibility transform reduces to a sign flip + unary add.
  * K2 is a Kronecker product of 1-D Gaussians; applied fully on-chip as
    Kronecker-factor matmuls plus two PE transposes.
  * K1 = exp(f.f' -.5|f|^2 -.5|f'|^2): BOTH quadratic terms ride as
    hi/lo-split bf16 rows of the d^2 matmul (K=19), so the exp ACT needs
    no bias and runs 2 tiles per instruction ((N+352)/1.2 ns amortized) -
    the ACT engine streams the whole materialization back-to-back.
  * K1 block rowsum partials via per-2-tile DVE free-dim reduces on the
    fp8 A tiles (DVE is idle during materialization), one 32KB AllGather
    of the per-core partial grids, then a 7-add tree.  s1n (own-block
    columns) is extracted with a per-core one-hot selector matmul.
  * Per-iteration K1 matvec streams A as the fp8 MOVING operand with
    perf_mode=DoubleRow (2 fp8 rows/lane/cycle, measured 216ns per
    [128,2,512] matmul) against a tiny stationary q [128,2,4].
  * ONE 8KB AllGather per iteration in partition-major block layout: the
    gathered buffer lands in exactly the SBUF layout the matvec and the
    separable pipeline need - no transposes on the consumer side.  A 16x
    scale folded into s1m keeps the fp8 q operand in normal range.

Sharding: voxel dim N row-blocked over 8 cores.  Each core materializes and
keeps its [8192 x 1024] column-block of K1 (fp8, 8 MB) in SBUF.
"""

import numpy as np
import ml_dtypes

BF16 = ml_dtypes.bfloat16

B, C, X, Y, Z = 1, 4, 32, 16, 16
N = X * Y * Z            # 8192
P = 128                  # SBUF partitions
NCORES = 8
NB = N // NCORES         # 1024 rows per core
TM = N // P              # 64 m-tiles
TB = NB // P             # 8 block tiles
QS = 16                  # padded per-tile stride of the fp8 q operand
ALPHA = 5.0              # = BETA = GAMMA in this problem
NUM_ITER = 5
W_1 = 1.0
W_2 = 1.0
LN16 = float(np.log(16.0))

_CACHE = {}
DUMMY_AG = True


def _split_hi_lo(v):
    hi = v.astype(BF16).astype(np.float32)
    lo = (v - hi).astype(BF16).astype(np.float32)
    return hi, lo


def _to_block_layout(v_nc):
    """[N, C] -> [NCORES, 128, TB*C] block-p-major device layout."""
    # n = k*NB + tt*128 + p
    return (
        v_nc.reshape(NCORES, TB, P, C).transpose(0, 2, 1, 3).reshape(NCORES, P, TB * C)
    )


def _host_constants(log_unary, features_pairwise):
    """All host-side numpy prep: layouts, constants, initial softmax."""
    lu = np.asarray(log_unary, np.float32).reshape(C, N)
    img = np.asarray(features_pairwise, np.float32).reshape(2, N)

    gx, gy, gz = np.meshgrid(
        np.arange(X), np.arange(Y), np.arange(Z), indexing="ij"
    )
    spatial = np.stack([gx, gy, gz], 0).astype(np.float32).reshape(3, N)

    f1 = np.concatenate([spatial, img], 0) / ALPHA      # [5, N]
    sq1 = (f1 * f1).sum(0)                              # [N]
    bcol = -0.5 * sq1

    f_hi, f_lo = _split_hi_lo(f1)
    b_hi, b_lo = _split_hi_lo(bcol)
    ones = np.ones((1, N), np.float32)
    # row r of lhs multiplies row r of rhs; sum over rows gives the full
    # exponent f_m.f_n - .5|f_n|^2 - .5|f_m|^2 (no ACT bias needed)
    lhs_rows = np.concatenate(
        [f_hi, f_lo, f_hi, ones, ones, b_hi[None], b_lo[None]], 0
    ).astype(BF16)                                      # [19, N]
    rhs_rows = np.concatenate(
        [f_hi, f_hi, f_lo, b_hi[None], b_lo[None], ones, ones], 0
    ).astype(BF16)                                      # [19, NB]

    # initial q0 = softmax(lu), shipped in the AllGather block layout
    # (partition-major: flat index = k*P*TB*C + p*TB*C + tt*C + c)
    e = np.exp(lu - lu.max(0, keepdims=True))
    q0 = (e / e.sum(0, keepdims=True)).T                # [N, 4]
    q0_blk = _to_block_layout(q0).reshape(-1).astype(BF16)

    # separable spatial kernel, normalization + W_2 folded into factors
    def g1d(n):
        a = np.arange(n, dtype=np.float32) / ALPHA
        return np.exp(-0.5 * (a[:, None] - a[None, :]) ** 2)

    Gx, Gy, Gz = g1d(X), g1d(Y), g1d(Z)
    gxp = Gx * (Gx.sum(1) ** -0.5)[:, None] * (Gx.sum(1) ** -0.5)[None, :]
    gyp = Gy * (Gy.sum(1) ** -0.5)[:, None] * (Gy.sum(1) ** -0.5)[None, :]
    gzp = Gz * (Gz.sum(1) ** -0.5)[:, None] * (Gz.sum(1) ** -0.5)[None, :]
    gxp *= W_2

    # Kronecker-factor constants for the on-chip pipeline
    kz = np.kron(np.eye(8, dtype=np.float32), gzp)             # [128, 128]
    ky = np.zeros((P, 4 * P), np.float32)                      # [(h*2+h')*128]
    for h in range(2):
        for hp in range(2):
            blk = np.kron(gyp[h * 8 : (h + 1) * 8, hp * 8 : (hp + 1) * 8],
                          np.eye(16, dtype=np.float32))
            ky[:, (h * 2 + hp) * P : (h * 2 + hp + 1) * P] = blk
    identity = np.eye(P, dtype=np.float32)

    lut_all = _to_block_layout(lu.T)                           # [8, 128, 32]

    in_maps = []
    for k in range(NCORES):
        blk = slice(k * NB, (k + 1) * NB)
        kx = np.kron(gxp[:, 4 * k : 4 * k + 4], np.eye(C, dtype=np.float32))
        ek = np.zeros((TM, TB), np.float32)
        for tt in range(TB):
            ek[k * TB + tt, tt] = 1.0
        in_maps.append(
            {
                "lhs_rows": np.ascontiguousarray(lhs_rows),
                "rhs_rows": np.ascontiguousarray(rhs_rows[:, blk]),
                "lut": np.ascontiguousarray(lut_all[k]),
                "q0": q0_blk,
                "kz": kz.astype(BF16),
                "ky": ky.astype(BF16),
                "kx": kx.astype(BF16),                         # [128, 16]
                "idb": identity.astype(BF16),
                "idf": identity,
                "ek": ek,
                "lncst": np.tile(
                    np.array([[LN16, -LN16]], np.float32), (P, 1)
                ),
            }
        )
    return in_maps


def _build_program():
    """Build the SPMD Bass/Tile program (same NEFF on all 8 cores)."""
    import concourse.bacc as bacc
    import concourse.mybir as mybir
    import concourse.tile as tile

    f32 = mybir.dt.float32
    bf16 = mybir.dt.bfloat16
    fp8 = mybir.dt.float8e4
    AF = mybir.ActivationFunctionType
    ln_func = getattr(AF, "Ln", None) or getattr(AF, "Log")
    DR = mybir.MatmulPerfMode.DoubleRow
    RG = [list(range(NCORES))]

    nc = bacc.Bacc(
        "TRN2", target_bir_lowering=False, debug=False, num_devices=NCORES
    )

    # I/O
    lhs_rows = nc.dram_tensor("lhs_rows", [19, N], bf16, kind="ExternalInput")
    rhs_rows = nc.dram_tensor("rhs_rows", [19, NB], bf16, kind="ExternalInput")
    lut_in = nc.dram_tensor("lut", [P, TB * C], f32, kind="ExternalInput")
    q0_in = nc.dram_tensor("q0", [NCORES * P * TB * C], bf16, kind="ExternalInput")
    kz_in = nc.dram_tensor("kz", [P, P], bf16, kind="ExternalInput")
    ky_in = nc.dram_tensor("ky", [P, 4 * P], bf16, kind="ExternalInput")
    kx_in = nc.dram_tensor("kx", [P, 4 * C], bf16, kind="ExternalInput")
    idb_in = nc.dram_tensor("idb", [P, P], bf16, kind="ExternalInput")
    idf_in = nc.dram_tensor("idf", [P, P], f32, kind="ExternalInput")
    ek_in = nc.dram_tensor("ek", [TM, TB], f32, kind="ExternalInput")
    lncst_in = nc.dram_tensor("lncst", [P, 2], f32, kind="ExternalInput")
    qout = nc.dram_tensor("qout", [P, TB * C], f32, kind="ExternalOutput")

    with tile.TileContext(nc) as tc:
        with (
            tc.tile_pool(name="const", bufs=1) as cp,
            tc.tile_pool(name="dram", bufs=1, space="DRAM") as dp,
        ):
            # ---- persistent SBUF tensors ----
            A_sb = cp.tile([P, TM * NB], fp8, name="A_sb")        # 64 KB/part
            lhsr_sb = cp.tile([19, N], bf16, name="lhsr_sb")
            rhsr_sb = cp.tile([19, NB], bf16, name="rhsr_sb")
            lut_sb = cp.tile([P, TB * C], f32, name="lut_sb")
            prs = cp.tile([P, TM], f32, name="prs")
            rs8_sb = cp.tile([P, NCORES * TM], f32, name="rs8_sb")
            rs_sb = cp.tile([P, TM], f32, name="rs_sb")
            kz_sb = cp.tile([P, P], bf16, name="kz_sb")
            ky_sb = cp.tile([P, 4 * P], bf16, name="ky_sb")
            kx_sb = cp.tile([P, 4 * C], bf16, name="kx_sb")
            idb_sb = cp.tile([P, P], bf16, name="idb_sb")
            idf_sb = cp.tile([P, P], f32, name="idf_sb")
            ek_sb = cp.tile([TM, TB], f32, name="ek_sb")
            lncst_sb = cp.tile([P, 2], f32, name="lncst_sb")

            A3 = A_sb[:].rearrange("p (t n) -> p t n", n=NB)

            # ---- DRAM scratch ----
            dum_in = dp.tile([512], f32, name="dum_in")
            dum_out = dp.tile([4096], f32, name="dum_out", addr_space="Shared")
            rs_loc = dp.tile([P * TM], f32, name="rs_loc")
            rs_all = dp.tile(
                [NCORES * P * TM], f32, name="rs_all", addr_space="Shared"
            )
            qag_in = [
                dp.tile([P * TB * C], bf16, name=f"qag_in{i}") for i in range(4)
            ]
            qag_out = [
                dp.tile(
                    [NCORES * P * TB * C], bf16, name=f"qag_out{i}",
                    addr_space="Shared",
                )
                for i in range(4)
            ]

            # ---- dummy collective first: hides the one-time global
            #      collective-entry barrier under materialization ----
            if DUMMY_AG:
                nc.sync.dma_start(
                    out=dum_in[:], in_=idf_in.ap().rearrange("p q -> (p q)")[0:512]
                )
                nc.gpsimd.collective_compute(
                    "AllGather",
                    mybir.AluOpType.bypass,
                    replica_groups=RG,
                    ins=[dum_in[:]],
                    outs=[dum_out[:]],
                )

            # ---- load constants; d^2 inputs first, with a small head
            #      chunk of lhs so the first matmuls start early ----
            nc.sync.dma_start(
                out=rhsr_sb[:, 0:512], in_=rhs_rows.ap()[:, 0:512]
            )
            nc.sync.dma_start(
                out=lhsr_sb[:, 0:256], in_=lhs_rows.ap()[:, 0:256]
            )
            nc.sync.dma_start(
                out=rhsr_sb[:, 512:1024], in_=rhs_rows.ap()[:, 512:1024]
            )
            nc.sync.dma_start(
                out=lhsr_sb[:, 256:2048], in_=lhs_rows.ap()[:, 256:2048]
            )
            for ch in range(1, 4):
                nc.sync.dma_start(
                    out=lhsr_sb[:, ch * 2048 : (ch + 1) * 2048],
                    in_=lhs_rows.ap()[:, ch * 2048 : (ch + 1) * 2048],
                )
            nc.scalar.dma_start(out=lut_sb[:], in_=lut_in.ap())
            nc.scalar.dma_start(out=kz_sb[:], in_=kz_in.ap())
            nc.scalar.dma_start(out=ky_sb[:], in_=ky_in.ap())
            nc.scalar.dma_start(out=kx_sb[:], in_=kx_in.ap())
            nc.scalar.dma_start(out=idb_sb[:], in_=idb_in.ap())
            nc.scalar.dma_start(out=idf_sb[:], in_=idf_in.ap())
            nc.scalar.dma_start(out=ek_sb[:], in_=ek_in.ap())
            nc.scalar.dma_start(out=lncst_sb[:], in_=lncst_in.ap())
            warm_exp = cp.tile([P, 1], f32, name="warm_exp")
            nc.scalar.activation(warm_exp[:], lncst_sb[:, 0:1], AF.Exp)

            # ================= materialization of A = K1 block =============
            # d^2 matmul (full exponent incl. both quadratic terms) -> exp on
            # ACT, two tiles per instruction; block rowsum partials via DVE
            # free-dim reduces on the fp8 A tiles (DVE is otherwise idle).
            with tc.tile_pool(name="matps", bufs=2, space="PSUM") as matps:
                for tq in range(TM // 2):
                    ps = matps.tile([P, 2 * NB], f32, name="mat_ps", tag="mat")
                    for t2 in range(2):
                        t = 2 * tq + t2
                        for h in range(2):
                            nc.tensor.matmul(
                                ps[:, t2 * NB + h * 512 : t2 * NB + (h + 1) * 512],
                                lhsr_sb[:, t * P : (t + 1) * P],
                                rhsr_sb[:, h * 512 : (h + 1) * 512],
                                start=True,
                                stop=True,
                                skip_group_check=True,
                            )
                    nc.scalar.activation(
                        A_sb[:, 2 * tq * NB : (2 * tq + 2) * NB], ps[:], AF.Exp
                    )
                    nc.vector.reduce_sum(
                        prs[:, 2 * tq : 2 * tq + 2],
                        A_sb[
                            :, 2 * tq * NB : (2 * tq + 2) * NB
                        ].rearrange("p (t n) -> p t n", n=NB),
                        axis=mybir.AxisListType.X,
                    )

            # ---- iteration pools open early: iteration 0's q-load and
            #      separable pipeline are emitted BEFORE the rowsum
            #      distribution, so every engine FIFO has runnable iter-0
            #      work queued ahead of the instructions that block on the
            #      rowsum AllGather.
            with (
                tc.tile_pool(name="itp", bufs=2) as itp,
                tc.tile_pool(name="sep", bufs=1) as sepp,
                tc.tile_pool(name="qps", bufs=2, space="PSUM") as qpsp,
                tc.tile_pool(name="sps", bufs=3, space="PSUM") as spsp,
            ):
                def emit_qload(it, q_l):
                    qsrc = q0_in.ap() if it == 0 else qag_out[it - 1][:]
                    ql3 = q_l[:].rearrange("p (k tc) -> p k tc", k=NCORES)
                    src3 = qsrc.rearrange("(k p tc) -> p k tc", k=NCORES, p=P)
                    nc.sync.dma_start(out=ql3[:, 0:4, :], in_=src3[:, 0:4, :])
                    nc.scalar.dma_start(out=ql3[:, 4:8, :], in_=src3[:, 4:8, :])

                def emit_sep(q_l, inter):
                    # separable spatial kernel (K2), all on-chip
                    zp = spsp.tile([P, TM * C], f32, name="zp", tag="sep")
                    nc.tensor.matmul(
                        zp[:], kz_sb[:], q_l[:], start=True, stop=True
                    )
                    w1 = sepp.tile([P, TM * C], bf16, name="w1")
                    nc.vector.tensor_copy(w1[:], zp[:])
                    inter(0)

                    yp = spsp.tile([P, 2 * X * C], f32, name="yp", tag="sep")
                    w1r = w1[:].rearrange("p (x h c) -> p x h c", h=2, c=C)
                    for hp in range(2):
                        for h in range(2):
                            nc.tensor.matmul(
                                yp[:, hp * P : (hp + 1) * P],
                                ky_sb[:, (h * 2 + hp) * P : (h * 2 + hp + 1) * P],
                                w1r[:, :, h, :],
                                start=(h == 0),
                                stop=(h == 1),
                            )
                    w2 = sepp.tile([P, 2 * X * C], bf16, name="w2")
                    nc.vector.tensor_copy(w2[:], yp[:])
                    inter(1)

                    q2sb = sepp.tile([P, TB * C], f32, name="q2sb")
                    q2r = q2sb[:].rearrange("p (x h c) -> p x h c", h=2, c=C)
                    for hp in range(2):
                        tp1 = spsp.tile([P, P], bf16, name="tp1", tag="sep")
                        nc.tensor.transpose(
                            tp1[:], w2[:, hp * P : (hp + 1) * P], idb_sb[:]
                        )
                        tx = sepp.tile([P, P], bf16, name="tx", tag="tx")
                        nc.vector.tensor_copy(tx[:], tp1[:])
                        xp = spsp.tile([4 * C, P], f32, name="xp", tag="sep")
                        nc.tensor.matmul(
                            xp[:], kx_sb[:], tx[:], start=True, stop=True
                        )
                        sx = sepp.tile([4 * C, P], bf16, name="sx", tag="sx")
                        nc.vector.tensor_copy(sx[:], xp[:])
                        tp2 = spsp.tile([P, 4 * C], bf16, name="tp2", tag="sep")
                        nc.tensor.transpose(
                            tp2[:], sx[:], idb_sb[:4 * C, :4 * C]
                        )
                        nc.vector.tensor_copy(
                            q2r[:, :, hp, :],
                            tp2[:].rearrange("p (x c) -> p x c", c=C),
                        )
                        inter(2 + hp)
                    # logits base = lu + q2, off the critical path
                    lq2 = sepp.tile([P, TB * C], f32, name="lq2")
                    nc.vector.tensor_add(lq2[:], q2sb[:], lut_sb[:])
                    return lq2

                # -- iteration-0 prefetch (no AllGather dependency) --
                q_l0 = itp.tile([P, TM * C], bf16, name="q_l", tag="q_l")
                emit_qload(0, q_l0)

                # -- rowsum distribution: AllGather partial grids --
                nc.sync.dma_start(
                    out=rs_loc[:].rearrange("(p t) -> p t", t=TM), in_=prs[:]
                )
                nc.gpsimd.collective_compute(
                    "AllGather",
                    mybir.AluOpType.bypass,
                    replica_groups=RG,
                    ins=[rs_loc[:]],
                    outs=[rs_all[:]],
                )
                nc.scalar.dma_start(
                    out=rs8_sb[:].rearrange("p (j t) -> p j t", t=TM),
                    in_=rs_all[:].rearrange("(j p t) -> p j t", p=P, t=TM),
                )

                # iteration-0 separable pipeline fills the AllGather wait
                lq2_0 = emit_sep(q_l0, lambda s: None)

                # -- 7-add tree (emitted after the sep copies so the blocked
                #    adds don't head-of-line block the Vector FIFO) --
                r8 = rs8_sb[:].rearrange("p (j t) -> p j t", t=TM)
                nc.vector.tensor_add(r8[:, 0, :], r8[:, 0, :], r8[:, 1, :])
                nc.vector.tensor_add(r8[:, 2, :], r8[:, 2, :], r8[:, 3, :])
                nc.vector.tensor_add(r8[:, 4, :], r8[:, 4, :], r8[:, 5, :])
                nc.vector.tensor_add(r8[:, 6, :], r8[:, 6, :], r8[:, 7, :])
                nc.vector.tensor_add(r8[:, 0, :], r8[:, 0, :], r8[:, 2, :])
                nc.vector.tensor_add(r8[:, 4, :], r8[:, 4, :], r8[:, 6, :])
                nc.vector.tensor_add(rs_sb[:], r8[:, 0, :], r8[:, 4, :])

                # S1 = rs^-1/2 via exp(-0.5*ln(rs)); 16x folded into s1m,
                # 1/16 into s1n.  Ln/Ln then Exp/Exp: two table switches.
                s1m_raw = cp.tile([P, TM], f32, name="s1m_raw")
                s1m_1 = cp.tile([P, TM], bf16, name="s1m_1")
                nc.scalar.activation(s1m_raw[:], rs_sb[:], ln_func)

                # s1n: own-block columns of rs via the one-hot selector ek
                rtp = spsp.tile([TM, P], f32, name="rtp", tag="sep")
                nc.tensor.transpose(rtp[:], rs_sb[:], idf_sb[:])
                rsT_sb = cp.tile([TM, P], f32, name="rsT_sb")
                nc.vector.tensor_copy(rsT_sb[:], rtp[:])
                snT = spsp.tile([TB, P], f32, name="snT", tag="sep")
                nc.tensor.matmul(
                    snT[:], ek_sb[:], rsT_sb[:], start=True, stop=True
                )
                snT_sb = cp.tile([TB, P], f32, name="snT_sb")
                nc.vector.tensor_copy(snT_sb[:], snT[:])
                snr = spsp.tile([P, TB], f32, name="snr", tag="sep")
                nc.tensor.transpose(snr[:], snT_sb[:], idf_sb[:TB, :TB])
                s1n_raw = cp.tile([P, TB], f32, name="s1n_raw")
                s1n_1 = cp.tile([P, TB], f32, name="s1n_1")
                nc.scalar.activation(s1n_raw[:], snr[:], ln_func)
                nc.scalar.activation(
                    s1m_1[:], s1m_raw[:], AF.Exp, scale=-0.5,
                    bias=lncst_sb[:, 0:1],
                )
                nc.scalar.activation(
                    s1n_1[:], s1n_raw[:], AF.Exp, scale=-0.5,
                    bias=lncst_sb[:, 1:2],
                )
                s1m_rep = cp.tile([P, TM * C], bf16, name="s1m_rep")
                s1n_rep = cp.tile([P, TB * C], f32, name="s1n_rep")
                s1m_r3 = s1m_rep[:].rearrange("p (t c) -> p t c", c=C)
                s1n_r3 = s1n_rep[:].rearrange("p (t c) -> p t c", c=C)
                for c in range(C):
                    (nc.vector if c < 2 else nc.gpsimd).tensor_copy(
                        s1m_r3[:, :, c], s1m_1[:]
                    )
                    (nc.vector if c < 2 else nc.gpsimd).tensor_copy(
                        s1n_r3[:, :, c], s1n_1[:]
                    )

                # ======================= iterations =======================
                for it in range(NUM_ITER):
                    last = it == NUM_ITER - 1

                    if it == 0:
                        q_l = q_l0
                    else:
                        q_l = itp.tile([P, TM * C], bf16, name="q_l", tag="q_l")
                        emit_qload(it, q_l)

                    # -- q_s = q_l * (16*s1m)  (fp8, padded stride-16) --
                    q_s = itp.tile([P, TM * QS], fp8, name="q_s", tag="q_s")
                    qs3 = q_s[:].rearrange("p (t c) -> p t c", c=QS)
                    ql4 = q_l[:].rearrange("p (t c) -> p t c", c=C)
                    sm4 = s1m_rep[:].rearrange("p (t c) -> p t c", c=C)
                    for k in range(NCORES):
                        sl = slice(k * TB, (k + 1) * TB)
                        eng = nc.vector if k % 2 == 0 else nc.gpsimd
                        eng.tensor_mul(
                            qs3[:, sl, 0:C], ql4[:, sl, :], sm4[:, sl, :]
                        )

                    q_ps = [
                        qpsp.tile(
                            [C, 512], f32, name=f"q_ps{h}", tag=f"qps{h}", bufs=1
                        )
                        for h in range(2)
                    ]

                    def matvec(h, tp_range):
                        for tp in tp_range:
                            nc.tensor.matmul(
                                q_ps[h][:],
                                qs3[:, 2 * tp : 2 * tp + 2, 0:C],
                                A3[:, 2 * tp : 2 * tp + 2, h * 512 : (h + 1) * 512],
                                start=(tp == 0),
                                stop=(tp == TM // 2 - 1),
                                perf_mode=DR,
                                skip_group_check=True,
                            )

                    if it == 0:
                        lq2 = lq2_0
                        matvec(0, range(0, 32))
                        matvec(1, range(0, 32))
                    else:
                        matvec(0, range(0, 16))

                        def inter(stage):
                            if stage == 0:
                                matvec(0, range(16, 32))
                            elif stage == 1:
                                matvec(1, range(0, 16))
                            elif stage == 2:
                                matvec(1, range(16, 24))
                            else:
                                matvec(1, range(24, 32))

                        lq2 = emit_sep(q_l, inter)

                    # -- transpose matvec result [4, NB] -> [128, TB*C] --
                    cn_sb = sepp.tile([C, NB], bf16, name="cn_sb")
                    for h in range(2):
                        nc.vector.tensor_copy(
                            cn_sb[:, h * 512 : (h + 1) * 512], q_ps[h][:]
                        )
                    qb_ps = spsp.tile(
                        [P, TB * C], bf16, name="qb_ps", tag="qb", bufs=1
                    )
                    for tt in range(TB):
                        nc.tensor.matmul(
                            qb_ps[:, tt * C : (tt + 1) * C],
                            cn_sb[:, tt * P : (tt + 1) * P],
                            idb_sb[:C, :C],
                            is_transpose=True,
                            start=(tt == 0),
                            stop=(tt == TB - 1),
                            skip_group_check=True,
                        )

                    # -- epilogue: logits = lu + S1n*u1 + q2 ; softmax --
                    u_sb = sepp.tile([P, TB * C], f32, name="u_sb")
                    nc.vector.tensor_mul(u_sb[:], qb_ps[:], s1n_rep[:])
                    nc.vector.tensor_add(u_sb[:], u_sb[:], lq2[:])
                    e_sb = sepp.tile([P, TB * C], f32, name="e_sb")
                    nc.scalar.activation(e_sb[:], u_sb[:], AF.Exp)
                    zs = sepp.tile([P, TB], f32, name="zs")
                    nc.vector.reduce_sum(
                        zs[:],
                        e_sb[:].rearrange("p (t c) -> p t c", c=C),
                        axis=mybir.AxisListType.X,
                    )
                    rz = sepp.tile([P, TB], f32, name="rz")
                    nc.vector.reciprocal(rz[:], zs[:])
                    rz_rep = sepp.tile([P, TB * C], f32, name="rz_rep")
                    rzr3 = rz_rep[:].rearrange("p (t c) -> p t c", c=C)
                    nc.vector.tensor_copy(rzr3[:, :, 0], rz[:])
                    nc.vector.tensor_copy(rzr3[:, :, 1], rz[:])
                    nc.gpsimd.tensor_copy(rzr3[:, :, 2], rz[:])
                    nc.gpsimd.tensor_copy(rzr3[:, :, 3], rz[:])
                    qn = sepp.tile(
                        [P, TB * C], f32 if last else bf16, name="qn",
                        tag="qn_f" if last else "qn_b",
                    )
                    nc.vector.tensor_mul(qn[:], e_sb[:], rz_rep[:])

                    if last:
                        nc.sync.dma_start(out=qout.ap(), in_=qn[:])
                    else:
                        nc.sync.dma_start(
                            out=qag_in[it][:].rearrange("(p tc) -> p tc", p=P),
                            in_=qn[:],
                        )
                        nc.gpsimd.collective_compute(
                            "AllGather",
                            mybir.AluOpType.bypass,
                            replica_groups=RG,
                            ins=[qag_in[it][:]],
                            outs=[qag_out[it][:]],
                        )

    nc.compile()
    return nc


def get_program():
    if "nc" not in _CACHE:
        _CACHE["nc"] = _build_program()
    return _CACHE["nc"]


def kernel(log_unary, features_pairwise, compatibility_weights):
    import concourse.bass_utils as bass_utils

    log_unary = np.asarray(log_unary)
    features_pairwise = np.asarray(features_pairwise)
    compatibility_weights = np.asarray(compatibility_weights)
    assert log_unary.shape == (B, C, X, Y, Z)
    assert features_pairwise.shape == (B, 2, X, Y, Z)
    potts = np.ones((C, C), np.float32) - np.eye(C, dtype=np.float32)
    assert np.abs(compatibility_weights.astype(np.float32) - potts).max() < 1e-5

    in_maps = _host_constants(log_unary, features_pairwise)
    nc = get_program()
    res = bass_utils.run_bass_kernel_spmd(
        nc, in_maps, core_ids=list(range(NCORES))
    )
    # qout[k] is [128, TB*C] block-p-major; invert the layout
    q = np.stack([res.results[k]["qout"] for k in range(NCORES)], 0)
    q = q.reshape(NCORES, P, TB, C).transpose(0, 2, 1, 3).reshape(N, C)
    out = q.T.reshape(B, C, X, Y, Z).astype(np.float32)
    return out


# revision 31
# speedup vs baseline: 1.0370x; 1.0345x over previous
"""Trainium2 Bass kernel for nn_CRF mean-field iteration (dense CRF, 5 iters).

Problem (hardcoded shapes): log_unary [1,4,32,16,16], features_pairwise
[1,2,32,16,16], compatibility = Potts (ones - eye).  N = 8192 voxels, C = 4.

Strategy
--------
Per reference, each iteration applies two dense [N,N] Gaussian kernels:
  K1 (bilateral, 5-D features) and K2 (spatial, 3-D features), both with
  rsqrt(rowsum) symmetric normalization, then a Potts compatibility
  transform and a softmax.

Key facts exploited:
  * Potts update: softmax over c is invariant to the per-voxel colsum term,
    so the compatibility transform reduces to a sign flip + unary add.
  * K2 is a Kronecker product of 1-D Gaussians; applied fully on-chip as
    Kronecker-factor matmuls plus two PE transposes.
  * K1 = exp(f.f' -.5|f|^2 -.5|f'|^2): BOTH quadratic terms ride as
    hi/lo-split bf16 rows of the d^2 matmul (K=19), so the exp ACT needs
    no bias and runs 2 tiles per instruction ((N+352)/1.2 ns amortized) -
    the ACT engine streams the whole materialization back-to-back.
  * K1 block rowsum partials via per-2-tile DVE free-dim reduces on the
    fp8 A tiles (DVE is idle during materialization), one 32KB AllGather
    of the per-core partial grids, then a 7-add tree.  s1n (own-block
    columns) is extracted with a per-core one-hot selector matmul.
  * Per-iteration K1 matvec streams A as the fp8 MOVING operand with
    perf_mode=DoubleRow (2 fp8 rows/lane/cycle, measured 216ns per
    [128,2,512] matmul) against a tiny stationary q [128,2,4].
  * ONE 8KB AllGather per iteration in partition-major block layout: the
    gathered buffer lands in exactly the SBUF layout the matvec and the
    separable pipeline need - no transposes on the consumer side.  A 16x
    scale folded into s1m keeps the fp8 q operand in normal range.

Sharding: voxel dim N row-blocked over 8 cores.  Each core materializes and
keeps its [8192 x 1024] column-block of K1 (fp8, 8 MB) in SBUF.
"""

import numpy as np
import ml_dtypes

BF16 = ml_dtypes.bfloat16

B, C, X, Y, Z = 1, 4, 32, 16, 16
N = X * Y * Z            # 8192
P = 128                  # SBUF partitions
NCORES = 8
NB = N // NCORES         # 1024 rows per core
TM = N // P              # 64 m-tiles
TB = NB // P             # 8 block tiles
QS = 16                  # padded per-tile stride of the fp8 q operand
ALPHA = 5.0              # = BETA = GAMMA in this problem
NUM_ITER = 5
W_1 = 1.0
W_2 = 1.0
LN16 = float(np.log(16.0))

_CACHE = {}
DUMMY_AG = True


def _split_hi_lo(v):
    hi = v.astype(BF16).astype(np.float32)
    lo = (v - hi).astype(BF16).astype(np.float32)
    return hi, lo


def _to_block_layout(v_nc):
    """[N, C] -> [NCORES, 128, TB*C] block-p-major device layout."""
    # n = k*NB + tt*128 + p
    return (
        v_nc.reshape(NCORES, TB, P, C).transpose(0, 2, 1, 3).reshape(NCORES, P, TB * C)
    )


def _host_constants(log_unary, features_pairwise):
    """All host-side numpy prep: layouts, constants, initial softmax."""
    lu = np.asarray(log_unary, np.float32).reshape(C, N)
    img = np.asarray(features_pairwise, np.float32).reshape(2, N)

    gx, gy, gz = np.meshgrid(
        np.arange(X), np.arange(Y), np.arange(Z), indexing="ij"
    )
    spatial = np.stack([gx, gy, gz], 0).astype(np.float32).reshape(3, N)

    f1 = np.concatenate([spatial, img], 0) / ALPHA      # [5, N]
    sq1 = (f1 * f1).sum(0)                              # [N]
    bcol = -0.5 * sq1

    f_hi, f_lo = _split_hi_lo(f1)
    b_hi, b_lo = _split_hi_lo(bcol)
    ones = np.ones((1, N), np.float32)
    # row r of lhs multiplies row r of rhs; sum over rows gives the full
    # exponent f_m.f_n - .5|f_n|^2 - .5|f_m|^2 (no ACT bias needed)
    lhs_rows = np.concatenate(
        [f_hi, f_lo, f_hi, ones, ones, b_hi[None], b_lo[None]], 0
    ).astype(BF16)                                      # [19, N]
    rhs_rows = np.concatenate(
        [f_hi, f_hi, f_lo, b_hi[None], b_lo[None], ones, ones], 0
    ).astype(BF16)                                      # [19, NB]

    # initial q0 = softmax(lu), shipped in the AllGather block layout
    # (partition-major: flat index = k*P*TB*C + p*TB*C + tt*C + c)
    e = np.exp(lu - lu.max(0, keepdims=True))
    q0 = (e / e.sum(0, keepdims=True)).T                # [N, 4]
    q0_blk = _to_block_layout(q0).reshape(-1).astype(BF16)

    # separable spatial kernel, normalization + W_2 folded into factors
    def g1d(n):
        a = np.arange(n, dtype=np.float32) / ALPHA
        return np.exp(-0.5 * (a[:, None] - a[None, :]) ** 2)

    Gx, Gy, Gz = g1d(X), g1d(Y), g1d(Z)
    gxp = Gx * (Gx.sum(1) ** -0.5)[:, None] * (Gx.sum(1) ** -0.5)[None, :]
    gyp = Gy * (Gy.sum(1) ** -0.5)[:, None] * (Gy.sum(1) ** -0.5)[None, :]
    gzp = Gz * (Gz.sum(1) ** -0.5)[:, None] * (Gz.sum(1) ** -0.5)[None, :]
    gxp *= W_2

    # Kronecker-factor constants for the on-chip pipeline
    kz = np.kron(np.eye(8, dtype=np.float32), gzp)             # [128, 128]
    ky = np.zeros((P, 4 * P), np.float32)                      # [(h*2+h')*128]
    for h in range(2):
        for hp in range(2):
            blk = np.kron(gyp[h * 8 : (h + 1) * 8, hp * 8 : (hp + 1) * 8],
                          np.eye(16, dtype=np.float32))
            ky[:, (h * 2 + hp) * P : (h * 2 + hp + 1) * P] = blk
    identity = np.eye(P, dtype=np.float32)

    lut_all = _to_block_layout(lu.T)                           # [8, 128, 32]

    in_maps = []
    for k in range(NCORES):
        blk = slice(k * NB, (k + 1) * NB)
        kx = np.kron(gxp[:, 4 * k : 4 * k + 4], np.eye(C, dtype=np.float32))
        ek = np.zeros((TM, TB), np.float32)
        for tt in range(TB):
            ek[k * TB + tt, tt] = 1.0
        in_maps.append(
            {
                "lhs_rows": np.ascontiguousarray(lhs_rows),
                "rhs_rows": np.ascontiguousarray(rhs_rows[:, blk]),
                "lut": np.ascontiguousarray(lut_all[k]),
                "q0": q0_blk,
                "kz": kz.astype(BF16),
                "ky": ky.astype(BF16),
                "kx": kx.astype(BF16),                         # [128, 16]
                "idb": identity.astype(BF16),
                "idf": identity,
                "ek": ek,
                "lncst": np.tile(
                    np.array([[LN16, -LN16]], np.float32), (P, 1)
                ),
            }
        )
    return in_maps


def _build_program():
    """Build the SPMD Bass/Tile program (same NEFF on all 8 cores)."""
    import concourse.bacc as bacc
    import concourse.mybir as mybir
    import concourse.tile as tile

    f32 = mybir.dt.float32
    bf16 = mybir.dt.bfloat16
    fp8 = mybir.dt.float8e4
    AF = mybir.ActivationFunctionType
    ln_func = getattr(AF, "Ln", None) or getattr(AF, "Log")
    DR = mybir.MatmulPerfMode.DoubleRow
    RG = [list(range(NCORES))]

    nc = bacc.Bacc(
        "TRN2", target_bir_lowering=False, debug=False, num_devices=NCORES
    )

    # I/O
    lhs_rows = nc.dram_tensor("lhs_rows", [19, N], bf16, kind="ExternalInput")
    rhs_rows = nc.dram_tensor("rhs_rows", [19, NB], bf16, kind="ExternalInput")
    lut_in = nc.dram_tensor("lut", [P, TB * C], f32, kind="ExternalInput")
    q0_in = nc.dram_tensor("q0", [NCORES * P * TB * C], bf16, kind="ExternalInput")
    kz_in = nc.dram_tensor("kz", [P, P], bf16, kind="ExternalInput")
    ky_in = nc.dram_tensor("ky", [P, 4 * P], bf16, kind="ExternalInput")
    kx_in = nc.dram_tensor("kx", [P, 4 * C], bf16, kind="ExternalInput")
    idb_in = nc.dram_tensor("idb", [P, P], bf16, kind="ExternalInput")
    idf_in = nc.dram_tensor("idf", [P, P], f32, kind="ExternalInput")
    ek_in = nc.dram_tensor("ek", [TM, TB], f32, kind="ExternalInput")
    lncst_in = nc.dram_tensor("lncst", [P, 2], f32, kind="ExternalInput")
    qout = nc.dram_tensor("qout", [P, TB * C], f32, kind="ExternalOutput")

    with tile.TileContext(nc) as tc:
        with (
            tc.tile_pool(name="const", bufs=1) as cp,
            tc.tile_pool(name="dram", bufs=1, space="DRAM") as dp,
        ):
            # ---- persistent SBUF tensors ----
            A_sb = cp.tile([P, TM * NB], fp8, name="A_sb")        # 64 KB/part
            lhsr_sb = cp.tile([19, N], bf16, name="lhsr_sb")
            rhsr_sb = cp.tile([19, NB], bf16, name="rhsr_sb")
            lut_sb = cp.tile([P, TB * C], f32, name="lut_sb")
            prs = cp.tile([P, TM], f32, name="prs")
            rs8_sb = cp.tile([P, NCORES * TM], f32, name="rs8_sb")
            rs_sb = cp.tile([P, TM], f32, name="rs_sb")
            kz_sb = cp.tile([P, P], bf16, name="kz_sb")
            ky_sb = cp.tile([P, 4 * P], bf16, name="ky_sb")
            kx_sb = cp.tile([P, 4 * C], bf16, name="kx_sb")
            idb_sb = cp.tile([P, P], bf16, name="idb_sb")
            idf_sb = cp.tile([P, P], f32, name="idf_sb")
            ek_sb = cp.tile([TM, TB], f32, name="ek_sb")
            lncst_sb = cp.tile([P, 2], f32, name="lncst_sb")

            A3 = A_sb[:].rearrange("p (t n) -> p t n", n=NB)

            # ---- DRAM scratch ----
            dum_in = dp.tile([512], f32, name="dum_in")
            dum_out = dp.tile([4096], f32, name="dum_out", addr_space="Shared")
            rs_loc = dp.tile([P * TM], f32, name="rs_loc")
            rs_all = dp.tile(
                [NCORES * P * TM], f32, name="rs_all", addr_space="Shared"
            )
            qag_in = [
                dp.tile([P * TB * C], bf16, name=f"qag_in{i}") for i in range(4)
            ]
            qag_out = [
                dp.tile(
                    [NCORES * P * TB * C], bf16, name=f"qag_out{i}",
                    addr_space="Shared",
                )
                for i in range(4)
            ]

            # ---- dummy collective first: hides the one-time global
            #      collective-entry barrier under materialization ----
            if DUMMY_AG:
                nc.sync.dma_start(
                    out=dum_in[:], in_=idf_in.ap().rearrange("p q -> (p q)")[0:512]
                )
                nc.gpsimd.collective_compute(
                    "AllGather",
                    mybir.AluOpType.bypass,
                    replica_groups=RG,
                    ins=[dum_in[:]],
                    outs=[dum_out[:]],
                )

            # ---- load constants; d^2 inputs first, with a small head
            #      chunk of lhs so the first matmuls start early ----
            nc.sync.dma_start(
                out=rhsr_sb[:, 0:512], in_=rhs_rows.ap()[:, 0:512]
            )
            nc.sync.dma_start(
                out=lhsr_sb[:, 0:256], in_=lhs_rows.ap()[:, 0:256]
            )
            nc.sync.dma_start(
                out=rhsr_sb[:, 512:1024], in_=rhs_rows.ap()[:, 512:1024]
            )
            nc.sync.dma_start(
                out=lhsr_sb[:, 256:2048], in_=lhs_rows.ap()[:, 256:2048]
            )
            for ch in range(1, 4):
                nc.sync.dma_start(
                    out=lhsr_sb[:, ch * 2048 : (ch + 1) * 2048],
                    in_=lhs_rows.ap()[:, ch * 2048 : (ch + 1) * 2048],
                )
            nc.scalar.dma_start(out=lncst_sb[:], in_=lncst_in.ap())
            warm_exp = cp.tile([P, 1], f32, name="warm_exp")
            nc.scalar.activation(warm_exp[:], lncst_sb[:, 0:1], AF.Exp)
            nc.scalar.dma_start(out=lut_sb[:], in_=lut_in.ap())
            nc.scalar.dma_start(out=kz_sb[:], in_=kz_in.ap())
            nc.scalar.dma_start(out=ky_sb[:], in_=ky_in.ap())
            nc.scalar.dma_start(out=kx_sb[:], in_=kx_in.ap())
            nc.scalar.dma_start(out=idb_sb[:], in_=idb_in.ap())
            nc.scalar.dma_start(out=idf_sb[:], in_=idf_in.ap())
            nc.scalar.dma_start(out=ek_sb[:], in_=ek_in.ap())

            # ================= materialization of A = K1 block =============
            # d^2 matmul (full exponent incl. both quadratic terms) -> exp on
            # ACT, two tiles per instruction; block rowsum partials via DVE
            # free-dim reduces on the fp8 A tiles (DVE is otherwise idle).
            with tc.tile_pool(name="matps", bufs=2, space="PSUM") as matps:
                for tq in range(TM // 2):
                    ps = matps.tile([P, 2 * NB], f32, name="mat_ps", tag="mat")
                    for t2 in range(2):
                        t = 2 * tq + t2
                        for h in range(2):
                            nc.tensor.matmul(
                                ps[:, t2 * NB + h * 512 : t2 * NB + (h + 1) * 512],
                                lhsr_sb[:, t * P : (t + 1) * P],
                                rhsr_sb[:, h * 512 : (h + 1) * 512],
                                start=True,
                                stop=True,
                                skip_group_check=True,
                            )
                    nc.scalar.activation(
                        A_sb[:, 2 * tq * NB : (2 * tq + 2) * NB], ps[:], AF.Exp
                    )
                    nc.vector.reduce_sum(
                        prs[:, 2 * tq : 2 * tq + 2],
                        A_sb[
                            :, 2 * tq * NB : (2 * tq + 2) * NB
                        ].rearrange("p (t n) -> p t n", n=NB),
                        axis=mybir.AxisListType.X,
                    )

            # ---- iteration pools open early: iteration 0's q-load and
            #      separable pipeline are emitted BEFORE the rowsum
            #      distribution, so every engine FIFO has runnable iter-0
            #      work queued ahead of the instructions that block on the
            #      rowsum AllGather.
            with (
                tc.tile_pool(name="itp", bufs=2) as itp,
                tc.tile_pool(name="sep", bufs=1) as sepp,
                tc.tile_pool(name="qps", bufs=2, space="PSUM") as qpsp,
                tc.tile_pool(name="sps", bufs=3, space="PSUM") as spsp,
            ):
                def emit_qload(it, q_l):
                    qsrc = q0_in.ap() if it == 0 else qag_out[it - 1][:]
                    ql3 = q_l[:].rearrange("p (k tc) -> p k tc", k=NCORES)
                    src3 = qsrc.rearrange("(k p tc) -> p k tc", k=NCORES, p=P)
                    nc.sync.dma_start(out=ql3[:, 0:4, :], in_=src3[:, 0:4, :])
                    nc.scalar.dma_start(out=ql3[:, 4:8, :], in_=src3[:, 4:8, :])

                def emit_sep(q_l, inter):
                    # separable spatial kernel (K2), all on-chip
                    zp = spsp.tile([P, TM * C], f32, name="zp", tag="sep")
                    nc.tensor.matmul(
                        zp[:], kz_sb[:], q_l[:], start=True, stop=True
                    )
                    w1 = sepp.tile([P, TM * C], bf16, name="w1")
                    nc.vector.tensor_copy(w1[:], zp[:])
                    inter(0)

                    yp = spsp.tile([P, 2 * X * C], f32, name="yp", tag="sep")
                    w1r = w1[:].rearrange("p (x h c) -> p x h c", h=2, c=C)
                    for hp in range(2):
                        for h in range(2):
                            nc.tensor.matmul(
                                yp[:, hp * P : (hp + 1) * P],
                                ky_sb[:, (h * 2 + hp) * P : (h * 2 + hp + 1) * P],
                                w1r[:, :, h, :],
                                start=(h == 0),
                                stop=(h == 1),
                            )
                    w2 = sepp.tile([P, 2 * X * C], bf16, name="w2")
                    nc.vector.tensor_copy(w2[:], yp[:])
                    inter(1)

                    q2sb = sepp.tile([P, TB * C], f32, name="q2sb")
                    q2r = q2sb[:].rearrange("p (x h c) -> p x h c", h=2, c=C)
                    for hp in range(2):
                        tp1 = spsp.tile([P, P], bf16, name="tp1", tag="sep")
                        nc.tensor.transpose(
                            tp1[:], w2[:, hp * P : (hp + 1) * P], idb_sb[:]
                        )
                        tx = sepp.tile([P, P], bf16, name="tx", tag="tx")
                        nc.vector.tensor_copy(tx[:], tp1[:])
                        xp = spsp.tile([4 * C, P], f32, name="xp", tag="sep")
                        nc.tensor.matmul(
                            xp[:], kx_sb[:], tx[:], start=True, stop=True
                        )
                        sx = sepp.tile([4 * C, P], bf16, name="sx", tag="sx")
                        nc.vector.tensor_copy(sx[:], xp[:])
                        tp2 = spsp.tile([P, 4 * C], bf16, name="tp2", tag="sep")
                        nc.tensor.transpose(
                            tp2[:], sx[:], idb_sb[:4 * C, :4 * C]
                        )
                        nc.vector.tensor_copy(
                            q2r[:, :, hp, :],
                            tp2[:].rearrange("p (x c) -> p x c", c=C),
                        )
                        inter(2 + hp)
                    # logits base = lu + q2, off the critical path
                    lq2 = sepp.tile([P, TB * C], f32, name="lq2")
                    nc.vector.tensor_add(lq2[:], q2sb[:], lut_sb[:])
                    return lq2

                # -- iteration-0 prefetch (no AllGather dependency) --
                q_l0 = itp.tile([P, TM * C], bf16, name="q_l", tag="q_l")
                emit_qload(0, q_l0)

                # -- rowsum distribution: AllGather partial grids --
                nc.sync.dma_start(
                    out=rs_loc[:].rearrange("(p t) -> p t", t=TM), in_=prs[:]
                )
                nc.gpsimd.collective_compute(
                    "AllGather",
                    mybir.AluOpType.bypass,
                    replica_groups=RG,
                    ins=[rs_loc[:]],
                    outs=[rs_all[:]],
                )
                nc.scalar.dma_start(
                    out=rs8_sb[:].rearrange("p (j t) -> p j t", t=TM),
                    in_=rs_all[:].rearrange("(j p t) -> p j t", p=P, t=TM),
                )

                # iteration-0 separable pipeline fills the AllGather wait
                lq2_0 = emit_sep(q_l0, lambda s: None)

                # -- 7-add tree (emitted after the sep copies so the blocked
                #    adds don't head-of-line block the Vector FIFO) --
                r8 = rs8_sb[:].rearrange("p (j t) -> p j t", t=TM)
                nc.vector.tensor_add(r8[:, 0, :], r8[:, 0, :], r8[:, 1, :])
                nc.vector.tensor_add(r8[:, 2, :], r8[:, 2, :], r8[:, 3, :])
                nc.vector.tensor_add(r8[:, 4, :], r8[:, 4, :], r8[:, 5, :])
                nc.vector.tensor_add(r8[:, 6, :], r8[:, 6, :], r8[:, 7, :])
                nc.vector.tensor_add(r8[:, 0, :], r8[:, 0, :], r8[:, 2, :])
                nc.vector.tensor_add(r8[:, 4, :], r8[:, 4, :], r8[:, 6, :])
                nc.vector.tensor_add(rs_sb[:], r8[:, 0, :], r8[:, 4, :])

                # S1 = rs^-1/2 via exp(-0.5*ln(rs)); 16x folded into s1m,
                # 1/16 into s1n.  Ln/Ln then Exp/Exp: two table switches.
                s1m_raw = cp.tile([P, TM], f32, name="s1m_raw")
                s1m_1 = cp.tile([P, TM], bf16, name="s1m_1")
                nc.scalar.activation(s1m_raw[:], rs_sb[:], ln_func)

                # s1n: own-block columns of rs via the one-hot selector ek
                rtp = spsp.tile([TM, P], f32, name="rtp", tag="sep")
                nc.tensor.transpose(rtp[:], rs_sb[:], idf_sb[:])
                rsT_sb = cp.tile([TM, P], f32, name="rsT_sb")
                nc.vector.tensor_copy(rsT_sb[:], rtp[:])
                snT = spsp.tile([TB, P], f32, name="snT", tag="sep")
                nc.tensor.matmul(
                    snT[:], ek_sb[:], rsT_sb[:], start=True, stop=True
                )
                snT_sb = cp.tile([TB, P], f32, name="snT_sb")
                nc.vector.tensor_copy(snT_sb[:], snT[:])
                snr = spsp.tile([P, TB], f32, name="snr", tag="sep")
                nc.tensor.transpose(snr[:], snT_sb[:], idf_sb[:TB, :TB])
                s1n_raw = cp.tile([P, TB], f32, name="s1n_raw")
                s1n_1 = cp.tile([P, TB], f32, name="s1n_1")
                nc.scalar.activation(s1n_raw[:], snr[:], ln_func)
                nc.scalar.activation(
                    s1m_1[:], s1m_raw[:], AF.Exp, scale=-0.5,
                    bias=lncst_sb[:, 0:1],
                )
                nc.scalar.activation(
                    s1n_1[:], s1n_raw[:], AF.Exp, scale=-0.5,
                    bias=lncst_sb[:, 1:2],
                )
                s1m_rep = cp.tile([P, TM * C], bf16, name="s1m_rep")
                s1n_rep = cp.tile([P, TB * C], f32, name="s1n_rep")
                s1m_r3 = s1m_rep[:].rearrange("p (t c) -> p t c", c=C)
                s1n_r3 = s1n_rep[:].rearrange("p (t c) -> p t c", c=C)
                for c in range(C):
                    (nc.vector if c < 2 else nc.gpsimd).tensor_copy(
                        s1m_r3[:, :, c], s1m_1[:]
                    )
                    (nc.vector if c < 2 else nc.gpsimd).tensor_copy(
                        s1n_r3[:, :, c], s1n_1[:]
                    )

                # ======================= iterations =======================
                for it in range(NUM_ITER):
                    last = it == NUM_ITER - 1

                    if it == 0:
                        q_l = q_l0
                    else:
                        q_l = itp.tile([P, TM * C], bf16, name="q_l", tag="q_l")
                        emit_qload(it, q_l)

                    # -- q_s = q_l * (16*s1m)  (fp8, padded stride-16) --
                    q_s = itp.tile([P, TM * QS], fp8, name="q_s", tag="q_s")
                    qs3 = q_s[:].rearrange("p (t c) -> p t c", c=QS)
                    ql4 = q_l[:].rearrange("p (t c) -> p t c", c=C)
                    sm4 = s1m_rep[:].rearrange("p (t c) -> p t c", c=C)
                    for k in range(NCORES):
                        sl = slice(k * TB, (k + 1) * TB)
                        eng = nc.vector if k % 2 == 0 else nc.gpsimd
                        eng.tensor_mul(
                            qs3[:, sl, 0:C], ql4[:, sl, :], sm4[:, sl, :]
                        )

                    q_ps = [
                        qpsp.tile(
                            [C, 512], f32, name=f"q_ps{h}", tag=f"qps{h}", bufs=1
                        )
                        for h in range(2)
                    ]

                    def matvec(h, tp_range):
                        for tp in tp_range:
                            nc.tensor.matmul(
                                q_ps[h][:],
                                qs3[:, 2 * tp : 2 * tp + 2, 0:C],
                                A3[:, 2 * tp : 2 * tp + 2, h * 512 : (h + 1) * 512],
                                start=(tp == 0),
                                stop=(tp == TM // 2 - 1),
                                perf_mode=DR,
                                skip_group_check=True,
                            )

                    if it == 0:
                        lq2 = lq2_0
                        matvec(0, range(0, 32))
                        matvec(1, range(0, 32))
                    else:
                        matvec(0, range(0, 16))

                        def inter(stage):
                            if stage == 0:
                                matvec(0, range(16, 32))
                            elif stage == 1:
                                matvec(1, range(0, 16))
                            elif stage == 2:
                                matvec(1, range(16, 24))
                            else:
                                matvec(1, range(24, 32))

                        lq2 = emit_sep(q_l, inter)

                    # -- transpose matvec result [4, NB] -> [128, TB*C] --
                    cn_sb = sepp.tile([C, NB], bf16, name="cn_sb")
                    for h in range(2):
                        nc.vector.tensor_copy(
                            cn_sb[:, h * 512 : (h + 1) * 512], q_ps[h][:]
                        )
                    qb_ps = spsp.tile(
                        [P, TB * C], bf16, name="qb_ps", tag="qb", bufs=1
                    )
                    for tt in range(TB):
                        nc.tensor.matmul(
                            qb_ps[:, tt * C : (tt + 1) * C],
                            cn_sb[:, tt * P : (tt + 1) * P],
                            idb_sb[:C, :C],
                            is_transpose=True,
                            start=(tt == 0),
                            stop=(tt == TB - 1),
                            skip_group_check=True,
                        )

                    # -- epilogue: logits = lu + S1n*u1 + q2 ; softmax --
                    u_sb = sepp.tile([P, TB * C], f32, name="u_sb")
                    nc.vector.tensor_mul(u_sb[:], qb_ps[:], s1n_rep[:])
                    nc.vector.tensor_add(u_sb[:], u_sb[:], lq2[:])
                    e_sb = sepp.tile([P, TB * C], f32, name="e_sb")
                    nc.scalar.activation(e_sb[:], u_sb[:], AF.Exp)
                    zs = sepp.tile([P, TB], f32, name="zs")
                    nc.vector.reduce_sum(
                        zs[:],
                        e_sb[:].rearrange("p (t c) -> p t c", c=C),
                        axis=mybir.AxisListType.X,
                    )
                    rz = sepp.tile([P, TB], f32, name="rz")
                    nc.vector.reciprocal(rz[:], zs[:])
                    rz_rep = sepp.tile([P, TB * C], f32, name="rz_rep")
                    rzr3 = rz_rep[:].rearrange("p (t c) -> p t c", c=C)
                    nc.vector.tensor_copy(rzr3[:, :, 0], rz[:])
                    nc.vector.tensor_copy(rzr3[:, :, 1], rz[:])
                    nc.gpsimd.tensor_copy(rzr3[:, :, 2], rz[:])
                    nc.gpsimd.tensor_copy(rzr3[:, :, 3], rz[:])
                    qn = sepp.tile(
                        [P, TB * C], f32 if last else bf16, name="qn",
                        tag="qn_f" if last else "qn_b",
                    )
                    nc.vector.tensor_mul(qn[:], e_sb[:], rz_rep[:])

                    if last:
                        nc.sync.dma_start(out=qout.ap(), in_=qn[:])
                    else:
                        nc.sync.dma_start(
                            out=qag_in[it][:].rearrange("(p tc) -> p tc", p=P),
                            in_=qn[:],
                        )
                        nc.gpsimd.collective_compute(
                            "AllGather",
                            mybir.AluOpType.bypass,
                            replica_groups=RG,
                            ins=[qag_in[it][:]],
                            outs=[qag_out[it][:]],
                        )

    nc.compile()
    return nc


def get_program():
    if "nc" not in _CACHE:
        _CACHE["nc"] = _build_program()
    return _CACHE["nc"]


def kernel(log_unary, features_pairwise, compatibility_weights):
    import concourse.bass_utils as bass_utils

    log_unary = np.asarray(log_unary)
    features_pairwise = np.asarray(features_pairwise)
    compatibility_weights = np.asarray(compatibility_weights)
    assert log_unary.shape == (B, C, X, Y, Z)
    assert features_pairwise.shape == (B, 2, X, Y, Z)
    potts = np.ones((C, C), np.float32) - np.eye(C, dtype=np.float32)
    assert np.abs(compatibility_weights.astype(np.float32) - potts).max() < 1e-5

    in_maps = _host_constants(log_unary, features_pairwise)
    nc = get_program()
    res = bass_utils.run_bass_kernel_spmd(
        nc, in_maps, core_ids=list(range(NCORES))
    )
    # qout[k] is [128, TB*C] block-p-major; invert the layout
    q = np.stack([res.results[k]["qout"] for k in range(NCORES)], 0)
    q = q.reshape(NCORES, P, TB, C).transpose(0, 2, 1, 3).reshape(N, C)
    out = q.T.reshape(B, C, X, Y, Z).astype(np.float32)
    return out
